# revision 19
# baseline (speedup 1.0000x reference)
"""Causal multi-head attention block (b=4, s=2048, d=1024, 16 heads) on 8
Trainium2 NeuronCores.

Sharding: tensor-parallel over heads x data-parallel over batch.
Core c handles batch c//2 and head-half c%2 (8 of 16 heads):
  - QKV projection for its 8 heads over all 2048 tokens (bf16 matmuls,
    fp32 PSUM accumulation); V-bias is folded into the V projection as a
    rank-1 matmul (softmax identity: sum_k p'_k (v+b) = z + d*b, so the
    normalized result is z/d + b exactly)
  - causal attention in [k, q] score layout: scores for the even/odd head of
    a pair run concurrently in disjoint PE row-quadrants; softmax denominator
    comes for free from a ones-column appended to the V stationary; the causal
    mask is a precomputed 0/1 multiply on P' (DVE)
  - denominators of all 8 heads are staged at 32-aligned partitions of two
    [128, 512] tiles (DVE row copies) so the 1/d = exp(-ln d) pass is 4
    ScalarE ops per query chunk instead of 16; the per-head broadcast across
    partitions is one K=128 select-matmul per head pair
  - partial O projection over its 512-dim slice (+ b_o/2); one pairwise
    chunk-sized ReduceScatter(add) completes O and a sync-engine copy
    streams the reduced rows out while later chunks compute
  - projections of later token blocks and the previous chunk's O work are
    woven into the attention kc-loops as PE filler, released by a
    deadline key (qc, ht, kc) so each chunk starts without a serial
    projection burst.
"""

import sys

import numpy as np
import ml_dtypes

if "/opt/trn_rl_repo" not in sys.path:
    sys.path.insert(0, "/opt/trn_rl_repo")

from contextlib import ExitStack

import concourse.bass as bass
import concourse.tile as tile
from concourse import mybir
import concourse.bass_utils as bass_utils

P = 128
S = 2048          # sequence length
D = 1024          # d_model
DH = 64           # head dim
NHO = 8           # heads per core
DO = 512          # own d-model slice (8 heads * 64)
NW = 1536         # own qkv output cols (512 k + 512 q + 512 v)
FCH = D // P      # 8 feature chunks (contraction over d_model)
NQC = S // 512    # 4 query chunks of 512
dt = mybir.dt
AF = mybir.ActivationFunctionType

# Pin Exp and Ln to the one activation-table set that holds both
# (natural_log_exp_and_others); otherwise the table-load pass alternates
# between exp_and_others and natural_log, reloading tables mid-kernel.
import concourse.bacc as _bacc
from concourse import hw_specs as _hw_specs

_orig_gat = _hw_specs.get_activation_tables


def _gat_pinned(arch):
    t = _orig_gat(arch)
    out = {}
    for name, fns in t.items():
        if name != "natural_log_exp_and_others":
            fns = {f for f in fns if f not in (AF.Exp, AF.Ln)}
        out[name] = set(fns)
    return out


_bacc.get_activation_tables = _gat_pinned

_orig_iatl = _bacc.Bacc.insert_act_table_loads


def _iatl_once(self):
    _orig_iatl(self)
    seen = [False]
    for blk in self.main_func.blocks:
        new = []
        for inst in blk.instructions:
            if isinstance(inst, mybir.InstLoadActFuncSet):
                if seen[0]:
                    si = inst.sync_info
                    if si and (si.on_wait or si.on_update):
                        new.append(mybir.InstNoOp(
                            name=f"{inst.name}-tlstrip", engine=inst.engine,
                            ins=[], outs=[], sync_info=si))
                    continue
                seen[0] = True
            new.append(inst)
        blk.instructions[:] = new


_bacc.Bacc.insert_act_table_loads = _iatl_once


def _split_excess_waits(nc):
    """This walrus build allows 1 sync wait per instruction (2 for
    EventSemaphore); Tile's end-of-kernel drain can carry more. Move the
    extras onto preceding NoOps on the same engine."""
    for f in nc.m.functions:
        for bb in f.blocks:
            new_insts = []
            for inst in bb.instructions:
                si = inst.sync_info
                waits = list(si.on_wait) if si and si.on_wait else []
                cap = 2 if isinstance(inst, mybir.InstEventSemaphore) else 1
                if len(waits) > cap:
                    extras, keep = waits[:-cap], waits[-cap:]
                    for i, w in enumerate(extras):
                        new_insts.append(mybir.InstNoOp(
                            name=f"{inst.name}-wsplit{i}", engine=inst.engine,
                            ins=[], outs=[],
                            sync_info=mybir.SyncInfo(on_wait=[w], on_update=[])))
                    si.on_wait = keep
                new_insts.append(inst)
            bb.instructions[:] = new_insts


def _build():
    nc = bass.Bass("TRN2", target_bir_lowering=False, debug=False, num_devices=8)
    xt_d = nc.declare_dram_parameter("xt", [D, S], dt.bfloat16, isOutput=False)
    wqkv_d = nc.declare_dram_parameter("wqkv", [D, NW], dt.bfloat16, isOutput=False)
    wo_d = nc.declare_dram_parameter("wo", [DO, D], dt.bfloat16, isOutput=False)
    bqk_d = nc.declare_dram_parameter("bqk", [P, 8], dt.float32, isOutput=False)
    bv_d = nc.declare_dram_parameter("bvrow", [1, DO], dt.bfloat16, isOutput=False)
    bo_d = nc.declare_dram_parameter("bo", [1, D], dt.float32, isOutput=False)
    esel_d = nc.declare_dram_parameter("esel", [P, 256], dt.bfloat16, isOutput=False)
    out_d = nc.declare_dram_parameter("out", [S // 2, D], dt.bfloat16, isOutput=True)
    opart = nc.dram_tensor("opart", [S, D], dt.bfloat16)
    rsout = nc.dram_tensor("rsout", [S // 2, D], dt.bfloat16)

    with tile.TileContext(nc) as tc, ExitStack() as ctx:
        const = ctx.enter_context(tc.tile_pool(name="const", bufs=1))
        persist = ctx.enter_context(tc.tile_pool(name="persist", bufs=1))

        # ---- constants -------------------------------------------------
        bqk_sb = const.tile([P, 8], dt.float32, name="bqk", tag="bqk")
        nc.sync.dma_start(out=bqk_sb[:], in_=bqk_d[:])
        bv_row = const.tile([1, DO], dt.bfloat16, name="bv_row", tag="bv_row")
        nc.sync.dma_start(out=bv_row[:], in_=bv_d[:])
        bo_row = const.tile([1, D], dt.float32, name="bo_row", tag="bo_row")
        nc.sync.dma_start(out=bo_row[:], in_=bo_d[:])
        esel = const.tile([P, 256], dt.bfloat16, name="esel", tag="esel")
        nc.sync.dma_start(out=esel[:], in_=esel_d[:])
        bo_bc = const.tile([P, D], dt.float32, name="bo_bc", tag="bo_bc")
        ones_col = const.tile([1, P], dt.float32, name="ones_col", tag="ones_col")
        nc.vector.memset(ones_col[:], 1.0)
        ones_col_bf = const.tile([1, P], dt.bfloat16, name="ones_col_bf", tag="ones_col_bf")
        nc.vector.memset(ones_col_bf[:], 1.0)

        # causal P'-mask tiles: mask_i[p, f] = 1 if (f mod 512) - p - 128*i >= 0
        # (both 512-halves identical so one [128,1024] tile serves a full P' tile)
        ones_src = const.tile([P, 1024], dt.bfloat16, name="ones_src", tag="ones_src")
        nc.gpsimd.memset(ones_src[:], 1.0)
        cmask = []
        for i in range(4):
            cm = const.tile([P, 1024], dt.bfloat16, name=f"cmask{i}", tag=f"cmask{i}")
            nc.gpsimd.affine_select(
                cm[:], ones_src[:], pattern=[[0, 2], [1, 512]], base=-128 * i,
                channel_multiplier=-1, compare_op=mybir.AluOpType.is_ge, fill=0.0)
            cmask.append(cm)

        # ---- persistent activations -----------------------------------
        qT = [persist.tile([P, S], dt.bfloat16, name=f"qT{i}", tag=f"qT{i}") for i in range(4)]
        kT = [persist.tile([P, S], dt.bfloat16, name=f"kT{i}", tag=f"kT{i}") for i in range(4)]
        vv = [persist.tile([P, NHO * (DH + 1)], dt.bfloat16, name=f"vv{t}", tag=f"vv{t}")
              for t in range(S // P)]
        z_all = [persist.tile([P, S], dt.bfloat16, name=f"z{i}", tag=f"z{i}") for i in range(4)]
        wo_bf = [persist.tile([P, D], dt.bfloat16, name=f"wo{i}", tag=f"wo{i}") for i in range(4)]

        # ---- pools (PSUM: shared 2 + scores 4 + z 2 = 8 banks) --------
        ph1 = ctx.enter_context(tc.tile_pool(name="ph1", bufs=1))
        p_pool = ctx.enter_context(tc.tile_pool(name="p_pool", bufs=6))
        dn_pool = ctx.enter_context(tc.tile_pool(name="dn_pool", bufs=2))
        ost_pool = ctx.enter_context(tc.tile_pool(name="ost_pool", bufs=12))
        proj_ps = ctx.enter_context(tc.tile_pool(name="proj_ps", bufs=2, space="PSUM"))
        s_psp = ctx.enter_context(tc.tile_pool(name="s_psp", bufs=2, space="PSUM"))
        zro_psp = ctx.enter_context(tc.tile_pool(name="zro_psp", bufs=2, space="PSUM"))

        dsem = nc.alloc_semaphore("dsem")
        csem = nc.alloc_semaphore("csem")
        d2sem = nc.alloc_semaphore("d2sem")
        n_odma = [0]

        # ---- input loads: split across both HWDGE rings (sync + scalar),
        # highest-priority first so attention can start early.
        xt_bf = [ph1.tile([P, S], dt.bfloat16, name=f"xt{f}", tag=f"xt{f}") for f in range(FCH)]
        wq_bf = [ph1.tile([P, NW], dt.bfloat16, name=f"wq{f}", tag=f"wq{f}") for f in range(FCH)]

        _eng = [nc.sync, nc.scalar]

        # K weight cols (0:512) + x cols 0:1024 per feature chunk
        for f in range(FCH):
            _eng[f % 2].dma_start(
                out=wq_bf[f][:, 0:512], in_=wqkv_d[f * P:(f + 1) * P, 0:512])
            _eng[(f + 1) % 2].dma_start(
                out=xt_bf[f][:, 0:1024], in_=xt_d[f * P:(f + 1) * P, 0:1024])
        # Q weight cols next (the first scores need K and Q; V only gates
        # the first PV matmul so it can trail)
        for f in range(FCH):
            _eng[f % 2].dma_start(
                out=wq_bf[f][:, 512:1024], in_=wqkv_d[f * P:(f + 1) * P, 512:1024])
        for f in range(FCH):
            _eng[f % 2].dma_start(
                out=wq_bf[f][:, 1024:1536], in_=wqkv_d[f * P:(f + 1) * P, 1024:1536])
        # x cols 1024:2048
        for f in range(FCH):
            _eng[f % 2].dma_start(
                out=xt_bf[f][:, 1024:2048], in_=xt_d[f * P:(f + 1) * P, 1024:2048])
        for dc in range(4):
            _eng[dc % 2].dma_start(out=wo_bf[dc][:], in_=wo_d[dc * P:(dc + 1) * P, :])

        # scrub the P' pool once: restricted diagonal exps leave columns
        # [0:c0] unwritten, and the full-width mask multiply would turn
        # uninitialized-SBUF NaN/Inf into NaN (NaN*0). gpsimd is idle here.
        for _ in range(6):
            pt0 = p_pool.tile([P, 1024], dt.bfloat16, name="pt", tag="pt")
            nc.gpsimd.memset(pt0[:], 0.0)

        # broadcast b_o to all partitions via a K=1 matmul (one-time)
        for half in range(2):
            bps = proj_ps.tile([P, 512], dt.float32, name="bps", tag="ps")
            nc.tensor.matmul(
                bps[:], lhsT=ones_col[:],
                rhs=bo_row[0:1, half * 512:(half + 1) * 512],
                start=True, stop=True)
            nc.vector.tensor_copy(bo_bc[:, half * 512:(half + 1) * 512], bps[:])

        # ---- projection thunks, released by deadline (qc, ht, kc) ------
        def kq_one(base, n, t, bias_off, dst):
            def f():
                ps = proj_ps.tile([P, 512], dt.float32, name="ps", tag="ps")
                for fc in range(FCH):
                    nc.tensor.matmul(
                        ps[:], lhsT=wq_bf[fc][:, base + n * P:base + (n + 1) * P],
                        rhs=xt_bf[fc][:, t * 512:(t + 1) * 512],
                        start=(fc == 0), stop=(fc == FCH - 1))
                nc.vector.tensor_scalar_add(
                    dst[n][:, t * 512:(t + 1) * 512], ps[:],
                    bqk_sb[:, bias_off + n:bias_off + n + 1])
            return f

        def v_one(t16):
            def f():
                ps = proj_ps.tile([P, 512], dt.float32, name="ps", tag="ps")
                for fc in range(FCH):
                    nc.tensor.matmul(
                        ps[:], lhsT=xt_bf[fc][:, t16 * P:(t16 + 1) * P],
                        rhs=wq_bf[fc][:, 1024:1536],
                        start=(fc == 0), stop=False)
                # rank-1 V-bias accumulation: every token row += b_v
                nc.tensor.matmul(
                    ps[:], lhsT=ones_col_bf[:], rhs=bv_row[:],
                    start=False, stop=True)
                vview = vv[t16][:].rearrange("p (h c) -> p h c", c=DH + 1)
                nc.vector.tensor_copy(
                    vview[:, :, 0:DH], ps[:].rearrange("p (h c) -> p h c", c=DH))
                nc.vector.memset(vview[:, :, DH:DH + 1], 1.0)
            return f

        # weight layout is K|Q|V; Q(t,n) is needed at (qc=t, ht=n, kc=0),
        # K(t,n) at the diagonal (qc=t, ht=n, kc=4t), V(t16) at
        # (qc=t16//4, ht=0, kc=t16)
        pending = []
        for t in range(4):
            for n in range(4):
                pending.append(((t, n, 0), kq_one(512, n, t, 0, qT)))
                pending.append(((t, n, 4 * t), kq_one(0, n, t, 4, kT)))
            for t16 in range(4 * t, 4 * t + 4):
                pending.append(((t16 // 4, 0, t16), v_one(t16)))
        pending.sort(key=lambda e: e[0])

        kcnt = [0]
        weave_o = []

        def attention_pairs(qc):
            qs = qc * 512
            n_kc = 4 * (qc + 1)
            # denominator staging: head h -> partition 32*(h%4) of dnt[h//4];
            # unused partitions are set to 1.0 so Ln/Exp never produce NaN
            # (the select-matmul would propagate NaN*0)
            dnt = [dn_pool.tile([P, 512], dt.float32, name=f"dnt{i}", tag=f"dnt{i}")
                   for i in range(2)]
            nc.vector.memset(dnt[0][:], 1.0)
            nc.vector.memset(dnt[1][:], 1.0)
            for ht in range(NHO // 2):
                # heads 2*ht (rows 0:64) and 2*ht+1 (rows 64:128) share the
                # kT/qT tile; their K=64 score matmuls target disjoint PE
                # row-quadrants and run concurrently
                z0 = zro_psp.tile([DH + 1, 512], dt.float32, name="zps0", tag="zro")
                z1 = zro_psp.tile([DH + 1, 512], dt.float32, name="zps1", tag="zro")
                for kc in range(n_kc):
                    now = (qc, ht, kc)
                    while pending and pending[0][0] <= now:
                        pending.pop(0)[1]()
                    di = kc - 4 * qc   # >=0 -> diagonal block
                    # keys in this block can only be attended by queries
                    # >= 128*di (mod 512): restrict scores+exp to the valid
                    # columns; the full-width mask multiply zeroes the rest
                    # (cmask columns < 128*di are all-zero), so PV stays
                    # full-width. di=1 is not worth the extra exp overhead.
                    c0 = 128 * di if di >= 2 else 0
                    s_ps = s_psp.tile([P, 1024], dt.float32, name="sps", tag="sps")
                    nc.tensor.matmul(
                        s_ps[:, c0:512],
                        lhsT=kT[ht][0:DH, kc * P:(kc + 1) * P],
                        rhs=qT[ht][0:DH, qs + c0:qs + 512],
                        start=True, stop=True)
                    nc.tensor.matmul(
                        s_ps[:, 512 + c0:1024],
                        lhsT=kT[ht][DH:P, kc * P:(kc + 1) * P],
                        rhs=qT[ht][DH:P, qs + c0:qs + 512],
                        start=True, stop=True)
                    p_t = p_pool.tile([P, 1024], dt.bfloat16, name="pt", tag="pt")
                    if c0:
                        nc.scalar.activation(
                            p_t[:, c0:512], s_ps[:, c0:512], AF.Exp, scale=0.125)
                        nc.scalar.activation(
                            p_t[:, 512 + c0:1024], s_ps[:, 512 + c0:1024],
                            AF.Exp, scale=0.125)
                    else:
                        nc.scalar.activation(p_t[:], s_ps[:], AF.Exp, scale=0.125)
                    if di >= 0:
                        # causal mask: zero P' where k > q (DVE multiply;
                        # gpsimd is reserved for collective sequencing)
                        nc.vector.tensor_tensor(
                            p_t[:], p_t[:], cmask[di][:], mybir.AluOpType.mult)
                    kcnt[0] += 1
                    if weave_o and kcnt[0] % 3 == 0:
                        # previous chunk's O-projection groups are light
                        # (~0.85us) PE filler between S' and PV
                        weave_o.pop(0)()
                    elif pending and kcnt[0] % 6 == 3:
                        # pop a projection group ahead of its deadline
                        pending.pop(0)[1]()
                    nc.tensor.matmul(
                        z0[:], lhsT=vv[kc][:, (2 * ht) * 65:(2 * ht) * 65 + 65],
                        rhs=p_t[:, 0:512],
                        start=(kc == 0), stop=(kc == n_kc - 1))
                    nc.tensor.matmul(
                        z1[:], lhsT=vv[kc][:, (2 * ht + 1) * 65:(2 * ht + 1) * 65 + 65],
                        rhs=p_t[:, 512:1024],
                        start=(kc == 0), stop=(kc == n_kc - 1))
                for hp, z_ps in ((0, z0), (DH, z1)):
                    head = 2 * ht + (1 if hp else 0)
                    slot = 32 * (head % 4)
                    # stage this head's softmax denominator row; copy
                    # unnormalized z out so the PSUM ring slot frees early
                    nc.vector.tensor_copy(
                        dnt[head // 4][slot:slot + 1, :], z_ps[DH:DH + 1, :])
                    nc.vector.tensor_copy(
                        z_all[ht][hp:hp + DH, qs:qs + 512], z_ps[0:DH, :])
            # batched reciprocal: 1/d = exp(-ln d), 4 ScalarE ops for all 8
            # heads (cost scales with free-dim per lane, not partition count;
            # Ln and Exp share one activation-table set)
            rcpt = []
            for i in range(2):
                lnt = dn_pool.tile([P, 512], dt.float32, name=f"lnt{i}", tag=f"lnt{i}")
                nc.scalar.activation(lnt[:], dnt[i][:], AF.Ln)
                rcp = dn_pool.tile([P, 512], dt.bfloat16, name=f"rcp{i}", tag=f"rcp{i}")
                nc.scalar.activation(rcp[:], lnt[:], AF.Exp, scale=-1.0)
                rcpt.append(rcp)
            for ht in range(NHO // 2):
                rbc = zro_psp.tile([P, 512], dt.float32, name="rbc", tag="zro")
                nc.tensor.matmul(
                    rbc[:], lhsT=esel[:, (ht % 2) * 128:(ht % 2) * 128 + 128],
                    rhs=rcpt[ht // 2][:], start=True, stop=True)
                zsl = z_all[ht][:, qs:qs + 512]
                nc.vector.tensor_tensor(zsl, zsl, rbc[:], mybir.AluOpType.mult)

        def o_group(qc, t4, no, osts):
            tok = qc * 512 + t4 * P
            ps = zro_psp.tile([P, 512], dt.float32, name="ops", tag="zro")
            for dc in range(4):
                nc.tensor.matmul(
                    ps[:], lhsT=z_all[dc][:, tok:tok + P],
                    rhs=wo_bf[dc][:, no * 512:(no + 1) * 512],
                    start=(dc == 0), stop=(dc == 3))
            ost = ost_pool.tile([P, 512], dt.bfloat16, name="ost", tag="ost")
            nc.vector.tensor_tensor(
                ost[:], ps[:], bo_bc[:, no * 512:(no + 1) * 512],
                mybir.AluOpType.add)
            osts.append((tok, no, ost))

        def o_crit(qc, osts):
            # DMA this chunk's partials to DRAM (manual semaphores, so inside
            # a critical; two engines so the 8 stores drain twice as fast),
            # then one chunk-sized ReduceScatter with the pair core while
            # later chunks compute; a gpsimd copy streams the reduced rows
            # out (gpsimd so the blocking csem wait never delays anything on
            # the sync ring)
            with tc.tile_critical():
                for i, (tok, no, ost) in enumerate(osts):
                    eng = nc.sync if i % 2 == 0 else nc.gpsimd
                    eng.dma_start(
                        out=opart[tok:tok + P, no * 512:(no + 1) * 512],
                        in_=ost[:]).then_inc(dsem, 16)
                    n_odma[0] += 1
                nc.gpsimd.wait_ge(dsem, 16 * n_odma[0])
                nc.gpsimd.collective_compute(
                    "ReduceScatter", mybir.AluOpType.add,
                    replica_groups=[[0, 1], [2, 3], [4, 5], [6, 7]],
                    ins=[opart[qc * 512:(qc + 1) * 512, :]],
                    outs=[rsout[qc * 256:(qc + 1) * 256, :]],
                ).then_inc(csem, 1)
            with tc.tile_critical():
                nc.gpsimd.wait_ge(csem, qc + 1)
                nc.gpsimd.dma_start(
                    out=out_d[qc * 256:(qc + 1) * 256, :],
                    in_=rsout[qc * 256:(qc + 1) * 256, :]).then_inc(d2sem, 16)

        for qc in range(NQC):
            attention_pairs(qc)
            while weave_o:
                # leftover O work of the previous chunk
                weave_o.pop(0)()
            osts = []
            thunks = []
            for t4 in range(4):
                thunks.append(lambda qc=qc, t4=t4: o_group(qc, t4, 0, osts))
                thunks.append(lambda qc=qc, t4=t4: o_group(qc, t4, 1, osts))
            thunks.append(lambda qc=qc: o_crit(qc, osts))
            if qc < NQC - 1:
                weave_o.extend(thunks)
            else:
                for th in thunks:
                    th()
        with tc.tile_critical():
            nc.gpsimd.wait_ge(d2sem, 16 * 4)
        while pending:
            pending.pop(0)[1]()

    _split_excess_waits(nc)
    return nc


_NC = {}


def _get_nc():
    if "nc" not in _NC:
        _NC["nc"] = _build()
    return _NC["nc"]


def _shard(inputs):
    x = np.ascontiguousarray(inputs["x"], dtype=np.float32)
    W_qkv = np.asarray(inputs["W_qkv"], dtype=np.float32)
    b_qkv = np.asarray(inputs["b_qkv"], dtype=np.float32)
    W_o = np.asarray(inputs["W_o"], dtype=np.float32)
    b_o = np.asarray(inputs["b_o"], dtype=np.float32)

    esel = np.zeros((P, 256), dtype=np.float32)
    for m in range(2):
        esel[32 * (2 * m), m * 128:m * 128 + 64] = 1.0
        esel[32 * (2 * m + 1), m * 128 + 64:m * 128 + 128] = 1.0

    in_maps = []
    for c in range(8):
        b, hh = c // 2, c % 2
        sl = slice(hh * DO, (hh + 1) * DO)
        wq = W_qkv[sl]
        wk = W_qkv[D + hh * DO:D + hh * DO + DO]
        wv = W_qkv[2 * D + hh * DO:2 * D + hh * DO + DO]
        wqkvT = np.ascontiguousarray(np.concatenate([wk, wq, wv], axis=0).T)
        bqk = np.ascontiguousarray(
            np.concatenate([b_qkv[hh * DO:hh * DO + DO],
                            b_qkv[D + hh * DO:D + hh * DO + DO]])
            .reshape(8, P).T)
        bvrow = np.ascontiguousarray(
            b_qkv[2 * D + hh * DO:2 * D + hh * DO + DO].reshape(1, DO))
        woT = np.ascontiguousarray(W_o.T[sl])
        in_maps.append({
            "xt": np.ascontiguousarray(x[b].T).astype(ml_dtypes.bfloat16),
            "wqkv": wqkvT.astype(ml_dtypes.bfloat16),
            "wo": woT.astype(ml_dtypes.bfloat16),
            "bqk": bqk,
            "bvrow": bvrow.astype(ml_dtypes.bfloat16),
            "esel": esel.astype(ml_dtypes.bfloat16),
            "bo": np.ascontiguousarray((0.5 * b_o).reshape(1, D)),
        })
    return in_maps


def _unshard(results, batch):
    out = np.empty((batch, S, D), dtype=np.float32)
    for b in range(batch):
        # chunk qc covers tokens [qc*512, (qc+1)*512); rank r of the pair
        # holds tokens [qc*512 + r*256, qc*512 + (r+1)*256) at out rows
        # [qc*256, (qc+1)*256)
        for qc in range(4):
            for r in range(2):
                out[b, qc * 512 + r * 256:qc * 512 + (r + 1) * 256] = \
                    results[2 * b + r]["out"][qc * 256:(qc + 1) * 256] \
                    .astype(np.float32)
    return out


def _run(inputs, trace=False, trace_kwargs=None):
    nc = _get_nc()
    in_maps = _shard(inputs)
    if trace:
        import types
        if "antenv.axon_hooks" not in sys.modules:
            mod = types.ModuleType("antenv.axon_hooks")
            _hook = [None]
            mod.set_axon_ntff_profile_hook = lambda h: _hook.__setitem__(0, h)
            mod.get_axon_ntff_profile_hook = lambda: _hook[0]
            sys.modules["antenv.axon_hooks"] = mod
            from trn_agent_boot.trn_boot import _ntff_profile_via_ctypes
            mod.set_axon_ntff_profile_hook(
                _ntff_profile_via_ctypes("/opt/axon/libaxon_pjrt.so"))
        bass_utils.upload_artifacts = lambda tmpdir: tmpdir
    res = bass_utils.run_bass_kernel_spmd(
        nc, in_maps, core_ids=list(range(8)), trace=trace,
        **(trace_kwargs or {}))
    out = _unshard(res.results, inputs["x"].shape[0])
    return out, res


def kernel(**inputs) -> np.ndarray:
    out, _ = _run(inputs, trace=False)
    return out


# revision 21
# speedup vs baseline: 1.0269x; 1.0269x over previous
"""Causal multi-head attention block (b=4, s=2048, d=1024, 16 heads) on 8
Trainium2 NeuronCores.

Sharding: tensor-parallel over heads x data-parallel over batch.
Core c handles batch c//2 and head-half c%2 (8 of 16 heads):
  - QKV projection for its 8 heads over all 2048 tokens (bf16 matmuls,
    fp32 PSUM accumulation); V-bias is folded into the V projection as a
    rank-1 matmul (softmax identity: sum_k p'_k (v+b) = z + d*b, so the
    normalized result is z/d + b exactly)
  - causal attention in [k, q] score layout: scores for the even/odd head of
    a pair run concurrently in disjoint PE row-quadrants; softmax denominator
    comes for free from a ones-column appended to the V stationary; the causal
    mask is a precomputed 0/1 multiply on P' (DVE)
  - denominators of all 8 heads are staged at 32-aligned partitions of two
    [128, 512] tiles (DVE row copies) so the 1/d = exp(-ln d) pass is 4
    ScalarE ops per query chunk instead of 16; the per-head broadcast across
    partitions is one K=128 select-matmul per head pair
  - partial O projection over its 512-dim slice (+ b_o/2); one pairwise
    chunk-sized ReduceScatter(add) completes O and a sync-engine copy
    streams the reduced rows out while later chunks compute
  - projections of later token blocks and the previous chunk's O work are
    woven into the attention kc-loops as PE filler, released by a
    deadline key (qc, ht, kc) so each chunk starts without a serial
    projection burst.
"""

import sys

import numpy as np
import ml_dtypes

if "/opt/trn_rl_repo" not in sys.path:
    sys.path.insert(0, "/opt/trn_rl_repo")

from contextlib import ExitStack

import concourse.bass as bass
import concourse.tile as tile
from concourse import mybir
import concourse.bass_utils as bass_utils

P = 128
S = 2048          # sequence length
D = 1024          # d_model
DH = 64           # head dim
NHO = 8           # heads per core
DO = 512          # own d-model slice (8 heads * 64)
NW = 1536         # own qkv output cols (512 k + 512 q + 512 v)
FCH = D // P      # 8 feature chunks (contraction over d_model)
NQC = S // 512    # 4 query chunks of 512
dt = mybir.dt
AF = mybir.ActivationFunctionType

# Pin Exp and Ln to the one activation-table set that holds both
# (natural_log_exp_and_others); otherwise the table-load pass alternates
# between exp_and_others and natural_log, reloading tables mid-kernel.
import concourse.bacc as _bacc
from concourse import hw_specs as _hw_specs

_orig_gat = _hw_specs.get_activation_tables


def _gat_pinned(arch):
    t = _orig_gat(arch)
    out = {}
    for name, fns in t.items():
        if name != "natural_log_exp_and_others":
            fns = {f for f in fns if f not in (AF.Exp, AF.Ln)}
        out[name] = set(fns)
    return out


_bacc.get_activation_tables = _gat_pinned

_orig_iatl = _bacc.Bacc.insert_act_table_loads


def _iatl_once(self):
    _orig_iatl(self)
    seen = [False]
    for blk in self.main_func.blocks:
        new = []
        for inst in blk.instructions:
            if isinstance(inst, mybir.InstLoadActFuncSet):
                if seen[0]:
                    si = inst.sync_info
                    if si and (si.on_wait or si.on_update):
                        new.append(mybir.InstNoOp(
                            name=f"{inst.name}-tlstrip", engine=inst.engine,
                            ins=[], outs=[], sync_info=si))
                    continue
                seen[0] = True
            new.append(inst)
        blk.instructions[:] = new


_bacc.Bacc.insert_act_table_loads = _iatl_once


def _split_excess_waits(nc):
    """This walrus build allows 1 sync wait per instruction (2 for
    EventSemaphore); Tile's end-of-kernel drain can carry more. Move the
    extras onto preceding NoOps on the same engine."""
    for f in nc.m.functions:
        for bb in f.blocks:
            new_insts = []
            for inst in bb.instructions:
                si = inst.sync_info
                waits = list(si.on_wait) if si and si.on_wait else []
                cap = 2 if isinstance(inst, mybir.InstEventSemaphore) else 1
                if len(waits) > cap:
                    extras, keep = waits[:-cap], waits[-cap:]
                    for i, w in enumerate(extras):
                        new_insts.append(mybir.InstNoOp(
                            name=f"{inst.name}-wsplit{i}", engine=inst.engine,
                            ins=[], outs=[],
                            sync_info=mybir.SyncInfo(on_wait=[w], on_update=[])))
                    si.on_wait = keep
                new_insts.append(inst)
            bb.instructions[:] = new_insts


def _build():
    nc = bass.Bass("TRN2", target_bir_lowering=False, debug=False, num_devices=8)
    xt_d = nc.declare_dram_parameter("xt", [D, S], dt.bfloat16, isOutput=False)
    wqkv_d = nc.declare_dram_parameter("wqkv", [D, NW], dt.bfloat16, isOutput=False)
    wo_d = nc.declare_dram_parameter("wo", [DO, D], dt.bfloat16, isOutput=False)
    bqk_d = nc.declare_dram_parameter("bqk", [P, 8], dt.float32, isOutput=False)
    bv_d = nc.declare_dram_parameter("bvrow", [1, DO], dt.bfloat16, isOutput=False)
    bo_d = nc.declare_dram_parameter("bo", [1, D], dt.float32, isOutput=False)
    esel_d = nc.declare_dram_parameter("esel", [P, 256], dt.bfloat16, isOutput=False)
    out_d = nc.declare_dram_parameter("out", [S // 2, D], dt.bfloat16, isOutput=True)
    opart = nc.dram_tensor("opart", [S, D], dt.bfloat16)
    rsout = nc.dram_tensor("rsout", [S // 2, D], dt.bfloat16)

    with tile.TileContext(nc) as tc, ExitStack() as ctx:
        const = ctx.enter_context(tc.tile_pool(name="const", bufs=1))
        persist = ctx.enter_context(tc.tile_pool(name="persist", bufs=1))

        # ---- constants -------------------------------------------------
        bqk_sb = const.tile([P, 8], dt.float32, name="bqk", tag="bqk")
        nc.gpsimd.dma_start(out=bqk_sb[:], in_=bqk_d[:])
        bv_row = const.tile([1, DO], dt.bfloat16, name="bv_row", tag="bv_row")
        nc.gpsimd.dma_start(out=bv_row[:], in_=bv_d[:])
        bo_row = const.tile([1, D], dt.float32, name="bo_row", tag="bo_row")
        nc.gpsimd.dma_start(out=bo_row[:], in_=bo_d[:])
        esel = const.tile([P, 256], dt.bfloat16, name="esel", tag="esel")
        nc.gpsimd.dma_start(out=esel[:], in_=esel_d[:])
        bo_bc = const.tile([P, D], dt.float32, name="bo_bc", tag="bo_bc")
        ones_col = const.tile([1, P], dt.float32, name="ones_col", tag="ones_col")
        nc.vector.memset(ones_col[:], 1.0)
        ones_col_bf = const.tile([1, P], dt.bfloat16, name="ones_col_bf", tag="ones_col_bf")
        nc.vector.memset(ones_col_bf[:], 1.0)

        # causal P'-mask tiles: mask_i[p, f] = 1 if (f mod 512) - p - 128*i >= 0
        # (both 512-halves identical so one [128,1024] tile serves a full P' tile)
        ones_src = const.tile([P, 1024], dt.bfloat16, name="ones_src", tag="ones_src")
        nc.gpsimd.memset(ones_src[:], 1.0)
        cmask = []
        for i in range(4):
            cm = const.tile([P, 1024], dt.bfloat16, name=f"cmask{i}", tag=f"cmask{i}")
            nc.gpsimd.affine_select(
                cm[:], ones_src[:], pattern=[[0, 2], [1, 512]], base=-128 * i,
                channel_multiplier=-1, compare_op=mybir.AluOpType.is_ge, fill=0.0)
            cmask.append(cm)

        # ---- persistent activations -----------------------------------
        qT = [persist.tile([P, S], dt.bfloat16, name=f"qT{i}", tag=f"qT{i}") for i in range(4)]
        kT = [persist.tile([P, S], dt.bfloat16, name=f"kT{i}", tag=f"kT{i}") for i in range(4)]
        vv = [persist.tile([P, NHO * (DH + 1)], dt.bfloat16, name=f"vv{t}", tag=f"vv{t}")
              for t in range(S // P)]
        z_all = [persist.tile([P, S], dt.bfloat16, name=f"z{i}", tag=f"z{i}") for i in range(4)]
        wo_bf = [persist.tile([P, D], dt.bfloat16, name=f"wo{i}", tag=f"wo{i}") for i in range(4)]

        # ---- pools (PSUM: shared 2 + scores 4 + z 2 = 8 banks) --------
        ph1 = ctx.enter_context(tc.tile_pool(name="ph1", bufs=1))
        p_pool = ctx.enter_context(tc.tile_pool(name="p_pool", bufs=6))
        dn_pool = ctx.enter_context(tc.tile_pool(name="dn_pool", bufs=2))
        ost_pool = ctx.enter_context(tc.tile_pool(name="ost_pool", bufs=12))
        proj_ps = ctx.enter_context(tc.tile_pool(name="proj_ps", bufs=2, space="PSUM"))
        s_psp = ctx.enter_context(tc.tile_pool(name="s_psp", bufs=2, space="PSUM"))
        zro_psp = ctx.enter_context(tc.tile_pool(name="zro_psp", bufs=2, space="PSUM"))

        dsem = nc.alloc_semaphore("dsem")
        csem = nc.alloc_semaphore("csem")
        d2sem = nc.alloc_semaphore("d2sem")
        n_odma = [0]

        # ---- input loads: split across both HWDGE rings (sync + scalar),
        # highest-priority first so attention can start early.
        xt_bf = [ph1.tile([P, S], dt.bfloat16, name=f"xt{f}", tag=f"xt{f}") for f in range(FCH)]
        wq_bf = [ph1.tile([P, NW], dt.bfloat16, name=f"wq{f}", tag=f"wq{f}") for f in range(FCH)]

        _eng = [nc.sync, nc.scalar]

        # phase-major trigger order (the ~0.6us per-DMA trigger cost on the
        # issuing queue is what gates the start, so each phase's 8 triggers
        # go out 4-per-ring back to back): K weights, x lo, Q, V, x hi, wo
        for f in range(FCH):
            _eng[f % 2].dma_start(
                out=wq_bf[f][:, 0:512], in_=wqkv_d[f * P:(f + 1) * P, 0:512])
        for f in range(FCH):
            _eng[f % 2].dma_start(
                out=xt_bf[f][:, 0:1024], in_=xt_d[f * P:(f + 1) * P, 0:1024])
        for f in range(FCH):
            _eng[f % 2].dma_start(
                out=wq_bf[f][:, 512:1024], in_=wqkv_d[f * P:(f + 1) * P, 512:1024])
        for f in range(FCH):
            _eng[f % 2].dma_start(
                out=wq_bf[f][:, 1024:1536], in_=wqkv_d[f * P:(f + 1) * P, 1024:1536])
        for f in range(FCH):
            _eng[f % 2].dma_start(
                out=xt_bf[f][:, 1024:2048], in_=xt_d[f * P:(f + 1) * P, 1024:2048])
        for dc in range(4):
            _eng[dc % 2].dma_start(out=wo_bf[dc][:], in_=wo_d[dc * P:(dc + 1) * P, :])

        # scrub the P' pool once: restricted diagonal exps leave columns
        # [0:c0] unwritten, and the full-width mask multiply would turn
        # uninitialized-SBUF NaN/Inf into NaN (NaN*0). gpsimd is idle here.
        for _ in range(6):
            pt0 = p_pool.tile([P, 1024], dt.bfloat16, name="pt", tag="pt")
            nc.gpsimd.memset(pt0[:], 0.0)

        # broadcast b_o to all partitions via a K=1 matmul (one-time)
        for half in range(2):
            bps = proj_ps.tile([P, 512], dt.float32, name="bps", tag="ps")
            nc.tensor.matmul(
                bps[:], lhsT=ones_col[:],
                rhs=bo_row[0:1, half * 512:(half + 1) * 512],
                start=True, stop=True)
            nc.vector.tensor_copy(bo_bc[:, half * 512:(half + 1) * 512], bps[:])

        # ---- projection thunks, released by deadline (qc, ht, kc) ------
        def kq_one(base, n, t, bias_off, dst):
            def f():
                ps = proj_ps.tile([P, 512], dt.float32, name="ps", tag="ps")
                for fc in range(FCH):
                    nc.tensor.matmul(
                        ps[:], lhsT=wq_bf[fc][:, base + n * P:base + (n + 1) * P],
                        rhs=xt_bf[fc][:, t * 512:(t + 1) * 512],
                        start=(fc == 0), stop=(fc == FCH - 1))
                nc.vector.tensor_scalar_add(
                    dst[n][:, t * 512:(t + 1) * 512], ps[:],
                    bqk_sb[:, bias_off + n:bias_off + n + 1])
            return f

        def v_one(t16):
            def f():
                ps = proj_ps.tile([P, 512], dt.float32, name="ps", tag="ps")
                for fc in range(FCH):
                    nc.tensor.matmul(
                        ps[:], lhsT=xt_bf[fc][:, t16 * P:(t16 + 1) * P],
                        rhs=wq_bf[fc][:, 1024:1536],
                        start=(fc == 0), stop=False)
                # rank-1 V-bias accumulation: every token row += b_v
                nc.tensor.matmul(
                    ps[:], lhsT=ones_col_bf[:], rhs=bv_row[:],
                    start=False, stop=True)
                vview = vv[t16][:].rearrange("p (h c) -> p h c", c=DH + 1)
                nc.vector.tensor_copy(
                    vview[:, :, 0:DH], ps[:].rearrange("p (h c) -> p h c", c=DH))
                nc.vector.memset(vview[:, :, DH:DH + 1], 1.0)
            return f

        # weight layout is K|Q|V; Q(t,n) is needed at (qc=t, ht=n, kc=0),
        # K(t,n) at the diagonal (qc=t, ht=n, kc=4t), V(t16) at
        # (qc=t16//4, ht=0, kc=t16)
        pending = []
        for t in range(4):
            for n in range(4):
                pending.append(((t, n, 0), kq_one(512, n, t, 0, qT)))
                pending.append(((t, n, 4 * t), kq_one(0, n, t, 4, kT)))
            for t16 in range(4 * t, 4 * t + 4):
                pending.append(((t16 // 4, 0, t16), v_one(t16)))
        pending.sort(key=lambda e: e[0])

        kcnt = [0]
        weave_o = []

        def attention_pairs(qc):
            qs = qc * 512
            n_kc = 4 * (qc + 1)
            # denominator staging: head h -> partition 32*(h%4) of dnt[h//4];
            # unused partitions are set to 1.0 so Ln/Exp never produce NaN
            # (the select-matmul would propagate NaN*0)
            dnt = [dn_pool.tile([P, 512], dt.float32, name=f"dnt{i}", tag=f"dnt{i}")
                   for i in range(2)]
            nc.vector.memset(dnt[0][:], 1.0)
            nc.vector.memset(dnt[1][:], 1.0)
            for ht in range(NHO // 2):
                # heads 2*ht (rows 0:64) and 2*ht+1 (rows 64:128) share the
                # kT/qT tile; their K=64 score matmuls target disjoint PE
                # row-quadrants and run concurrently
                z0 = zro_psp.tile([DH + 1, 512], dt.float32, name="zps0", tag="zro")
                z1 = zro_psp.tile([DH + 1, 512], dt.float32, name="zps1", tag="zro")
                for kc in range(n_kc):
                    now = (qc, ht, kc)
                    while pending and pending[0][0] <= now:
                        pending.pop(0)[1]()
                    di = kc - 4 * qc   # >=0 -> diagonal block
                    # keys in this block can only be attended by queries
                    # >= 128*di (mod 512): restrict scores+exp to the valid
                    # columns; the full-width mask multiply zeroes the rest
                    # (cmask columns < 128*di are all-zero), so PV stays
                    # full-width. di=1 is not worth the extra exp overhead.
                    c0 = 128 * di if di >= 2 else 0
                    s_ps = s_psp.tile([P, 1024], dt.float32, name="sps", tag="sps")
                    nc.tensor.matmul(
                        s_ps[:, c0:512],
                        lhsT=kT[ht][0:DH, kc * P:(kc + 1) * P],
                        rhs=qT[ht][0:DH, qs + c0:qs + 512],
                        start=True, stop=True)
                    nc.tensor.matmul(
                        s_ps[:, 512 + c0:1024],
                        lhsT=kT[ht][DH:P, kc * P:(kc + 1) * P],
                        rhs=qT[ht][DH:P, qs + c0:qs + 512],
                        start=True, stop=True)
                    p_t = p_pool.tile([P, 1024], dt.bfloat16, name="pt", tag="pt")
                    if c0:
                        nc.scalar.activation(
                            p_t[:, c0:512], s_ps[:, c0:512], AF.Exp, scale=0.125)
                        nc.scalar.activation(
                            p_t[:, 512 + c0:1024], s_ps[:, 512 + c0:1024],
                            AF.Exp, scale=0.125)
                    else:
                        nc.scalar.activation(p_t[:], s_ps[:], AF.Exp, scale=0.125)
                    if di >= 0:
                        # causal mask: zero P' where k > q (DVE multiply;
                        # gpsimd is reserved for collective sequencing)
                        nc.vector.tensor_tensor(
                            p_t[:], p_t[:], cmask[di][:], mybir.AluOpType.mult)
                    kcnt[0] += 1
                    if weave_o and kcnt[0] % 3 == 0:
                        # previous chunk's O-projection groups are light
                        # (~0.85us) PE filler between S' and PV
                        weave_o.pop(0)()
                    elif pending and kcnt[0] % 6 == 3:
                        # pop a projection group ahead of its deadline
                        pending.pop(0)[1]()
                    nc.tensor.matmul(
                        z0[:], lhsT=vv[kc][:, (2 * ht) * 65:(2 * ht) * 65 + 65],
                        rhs=p_t[:, 0:512],
                        start=(kc == 0), stop=(kc == n_kc - 1))
                    nc.tensor.matmul(
                        z1[:], lhsT=vv[kc][:, (2 * ht + 1) * 65:(2 * ht + 1) * 65 + 65],
                        rhs=p_t[:, 512:1024],
                        start=(kc == 0), stop=(kc == n_kc - 1))
                for hp, z_ps in ((0, z0), (DH, z1)):
                    head = 2 * ht + (1 if hp else 0)
                    slot = 32 * (head % 4)
                    # stage this head's softmax denominator row; copy
                    # unnormalized z out so the PSUM ring slot frees early
                    nc.vector.tensor_copy(
                        dnt[head // 4][slot:slot + 1, :], z_ps[DH:DH + 1, :])
                    nc.vector.tensor_copy(
                        z_all[ht][hp:hp + DH, qs:qs + 512], z_ps[0:DH, :])
            # batched reciprocal: 1/d = exp(-ln d), 4 ScalarE ops for all 8
            # heads (cost scales with free-dim per lane, not partition count;
            # Ln and Exp share one activation-table set)
            rcpt = []
            for i in range(2):
                lnt = dn_pool.tile([P, 512], dt.float32, name=f"lnt{i}", tag=f"lnt{i}")
                nc.scalar.activation(lnt[:], dnt[i][:], AF.Ln)
                rcp = dn_pool.tile([P, 512], dt.bfloat16, name=f"rcp{i}", tag=f"rcp{i}")
                nc.scalar.activation(rcp[:], lnt[:], AF.Exp, scale=-1.0)
                rcpt.append(rcp)
            for ht in range(NHO // 2):
                rbc = zro_psp.tile([P, 512], dt.float32, name="rbc", tag="zro")
                nc.tensor.matmul(
                    rbc[:], lhsT=esel[:, (ht % 2) * 128:(ht % 2) * 128 + 128],
                    rhs=rcpt[ht // 2][:], start=True, stop=True)
                zsl = z_all[ht][:, qs:qs + 512]
                nc.vector.tensor_tensor(zsl, zsl, rbc[:], mybir.AluOpType.mult)

        def o_group(qc, t4, no, osts):
            tok = qc * 512 + t4 * P
            ps = zro_psp.tile([P, 512], dt.float32, name="ops", tag="zro")
            for dc in range(4):
                nc.tensor.matmul(
                    ps[:], lhsT=z_all[dc][:, tok:tok + P],
                    rhs=wo_bf[dc][:, no * 512:(no + 1) * 512],
                    start=(dc == 0), stop=(dc == 3))
            ost = ost_pool.tile([P, 512], dt.bfloat16, name="ost", tag="ost")
            nc.vector.tensor_tensor(
                ost[:], ps[:], bo_bc[:, no * 512:(no + 1) * 512],
                mybir.AluOpType.add)
            osts.append((tok, no, ost))

        def o_crit(qc, osts):
            # DMA this chunk's partials to DRAM (manual semaphores, so inside
            # a critical; two engines so the 8 stores drain twice as fast),
            # then one chunk-sized ReduceScatter with the pair core while
            # later chunks compute; a gpsimd copy streams the reduced rows
            # out (gpsimd so the blocking csem wait never delays anything on
            # the sync ring)
            with tc.tile_critical():
                for i, (tok, no, ost) in enumerate(osts):
                    eng = nc.sync if i % 2 == 0 else nc.gpsimd
                    eng.dma_start(
                        out=opart[tok:tok + P, no * 512:(no + 1) * 512],
                        in_=ost[:]).then_inc(dsem, 16)
                    n_odma[0] += 1
                nc.gpsimd.wait_ge(dsem, 16 * n_odma[0])
                nc.gpsimd.collective_compute(
                    "ReduceScatter", mybir.AluOpType.add,
                    replica_groups=[[0, 1], [2, 3], [4, 5], [6, 7]],
                    ins=[opart[qc * 512:(qc + 1) * 512, :]],
                    outs=[rsout[qc * 256:(qc + 1) * 256, :]],
                ).then_inc(csem, 1)
            with tc.tile_critical():
                nc.gpsimd.wait_ge(csem, qc + 1)
                nc.gpsimd.dma_start(
                    out=out_d[qc * 256:(qc + 1) * 256, :],
                    in_=rsout[qc * 256:(qc + 1) * 256, :]).then_inc(d2sem, 16)

        for qc in range(NQC):
            attention_pairs(qc)
            while weave_o:
                # leftover O work of the previous chunk
                weave_o.pop(0)()
            osts = []
            thunks = []
            for t4 in range(4):
                thunks.append(lambda qc=qc, t4=t4: o_group(qc, t4, 0, osts))
                thunks.append(lambda qc=qc, t4=t4: o_group(qc, t4, 1, osts))
            thunks.append(lambda qc=qc: o_crit(qc, osts))
            if qc < NQC - 1:
                weave_o.extend(thunks)
            else:
                for th in thunks:
                    th()
        with tc.tile_critical():
            nc.gpsimd.wait_ge(d2sem, 16 * 4)
        while pending:
            pending.pop(0)[1]()

    _split_excess_waits(nc)
    return nc


_NC = {}


def _get_nc():
    if "nc" not in _NC:
        _NC["nc"] = _build()
    return _NC["nc"]


def _shard(inputs):
    x = np.ascontiguousarray(inputs["x"], dtype=np.float32)
    W_qkv = np.asarray(inputs["W_qkv"], dtype=np.float32)
    b_qkv = np.asarray(inputs["b_qkv"], dtype=np.float32)
    W_o = np.asarray(inputs["W_o"], dtype=np.float32)
    b_o = np.asarray(inputs["b_o"], dtype=np.float32)

    esel = np.zeros((P, 256), dtype=np.float32)
    for m in range(2):
        esel[32 * (2 * m), m * 128:m * 128 + 64] = 1.0
        esel[32 * (2 * m + 1), m * 128 + 64:m * 128 + 128] = 1.0

    in_maps = []
    for c in range(8):
        b, hh = c // 2, c % 2
        sl = slice(hh * DO, (hh + 1) * DO)
        wq = W_qkv[sl]
        wk = W_qkv[D + hh * DO:D + hh * DO + DO]
        wv = W_qkv[2 * D + hh * DO:2 * D + hh * DO + DO]
        wqkvT = np.ascontiguousarray(np.concatenate([wk, wq, wv], axis=0).T)
        bqk = np.ascontiguousarray(
            np.concatenate([b_qkv[hh * DO:hh * DO + DO],
                            b_qkv[D + hh * DO:D + hh * DO + DO]])
            .reshape(8, P).T)
        bvrow = np.ascontiguousarray(
            b_qkv[2 * D + hh * DO:2 * D + hh * DO + DO].reshape(1, DO))
        woT = np.ascontiguousarray(W_o.T[sl])
        in_maps.append({
            "xt": np.ascontiguousarray(x[b].T).astype(ml_dtypes.bfloat16),
            "wqkv": wqkvT.astype(ml_dtypes.bfloat16),
            "wo": woT.astype(ml_dtypes.bfloat16),
            "bqk": bqk,
            "bvrow": bvrow.astype(ml_dtypes.bfloat16),
            "esel": esel.astype(ml_dtypes.bfloat16),
            "bo": np.ascontiguousarray((0.5 * b_o).reshape(1, D)),
        })
    return in_maps


def _unshard(results, batch):
    out = np.empty((batch, S, D), dtype=np.float32)
    for b in range(batch):
        # chunk qc covers tokens [qc*512, (qc+1)*512); rank r of the pair
        # holds tokens [qc*512 + r*256, qc*512 + (r+1)*256) at out rows
        # [qc*256, (qc+1)*256)
        for qc in range(4):
            for r in range(2):
                out[b, qc * 512 + r * 256:qc * 512 + (r + 1) * 256] = \
                    results[2 * b + r]["out"][qc * 256:(qc + 1) * 256] \
                    .astype(np.float32)
    return out


def _run(inputs, trace=False, trace_kwargs=None):
    nc = _get_nc()
    in_maps = _shard(inputs)
    if trace:
        import types
        if "antenv.axon_hooks" not in sys.modules:
            mod = types.ModuleType("antenv.axon_hooks")
            _hook = [None]
            mod.set_axon_ntff_profile_hook = lambda h: _hook.__setitem__(0, h)
            mod.get_axon_ntff_profile_hook = lambda: _hook[0]
            sys.modules["antenv.axon_hooks"] = mod
            from trn_agent_boot.trn_boot import _ntff_profile_via_ctypes
            mod.set_axon_ntff_profile_hook(
                _ntff_profile_via_ctypes("/opt/axon/libaxon_pjrt.so"))
        bass_utils.upload_artifacts = lambda tmpdir: tmpdir
    res = bass_utils.run_bass_kernel_spmd(
        nc, in_maps, core_ids=list(range(8)), trace=trace,
        **(trace_kwargs or {}))
    out = _unshard(res.results, inputs["x"].shape[0])
    return out, res


def kernel(**inputs) -> np.ndarray:
    out, _ = _run(inputs, trace=False)
    return out
